# revision 5
# baseline (speedup 1.0000x reference)
"""Trainium2 Bass kernel for nn_CosineLoss: mean_i(1 - output[i, targets[i]]).

Strategy (data-parallel over the batch dim, 8 cores):
  - Core c owns rows [c*1024, (c+1)*1024) of `output`, staged in HBM as
    fp8(e4m3) -- 33.5 MB per core. Quantization error on the mean of 8192
    gathered N(0,1) values is ~4e-4 relative, 50x under the 2e-2 gate.
  - The gather uses InstDMAGatherAnt (gpsimd.dma_gather): ONE SWDGE
    instruction fetches up to 512 scattered 256B chunks. Its int16 index *
    256B stride addresses exactly 8.4 MB, so the 33.5 MB fp8 shard is
    covered by exactly 4 windows -> 4 instructions. This replaces the 8
    indirect-DMA instructions (128-descriptor HW cap each) whose serialized
    ~1.4us Q7 descriptor-generation dominated the baseline (11.3us of 24.6).
  - Each target (i, t) maps to flat byte addr a = i*32000 + t; window
    w = a >> 23, chunk row = (a >> 8) & 32767, byte pos = a & 255. Targets
    are bucketed per window (counts ~256 +- 16; capacity 512 with pad
    index 0 so num_idxs_reg stays a compile-time constant).
  - Within-chunk selection + partial sum on device: a host-built fp8
    one-hot mask (1.0 at each target's (slot, pos)) feeds a chained DVE
    tensor_tensor_reduce: accum[p] = sum_w reduce(gath_w * mask_w). One
    DVE pass per window, overlapped with later windows' descriptor gen.
  - Sync DMAs out accum [128,1] f32; host sums the 8x128 partials and
    returns 1 - total/8192.
  - idx tile is [16, n] int16 wrapped (index k at [k%16, k//16]) and
    replicated to all 128 partitions: each of the 8 Q7 cores reads its own
    16-partition copy (HW-verified; a [16, n] tile leaves cores 1-7 reading
    garbage).
"""

import numpy as np

from concourse import bacc, mybir
from concourse.bass_utils import run_bass_kernel_spmd

N = 8192
C = 32000
NCORES = 8
NL = N // NCORES  # 1024 rows per core
P = 128

SHARD_BYTES = NL * C  # fp8: 1 byte/elem = 32_768_000... padded below
CHUNK = 256  # dma_gather element size (bytes); min/alignment 256
ROWS_PER_WIN = 32768  # int16 index reach
WIN_BYTES = ROWS_PER_WIN * CHUNK  # 8_388_608
NWIN = 4  # ceil(NL*C / WIN_BYTES) = 3.906 -> 4 (padded shard 33.5 MB)
NIDX = 512  # per-window index capacity (mean 256, +16 sigma)
SLOTS = NIDX // 128  # dst free slots per window
ICOL = NIDX // 16  # idx tile columns per window

_NC_CACHE = {}


def _build():
    nc = bacc.Bacc("TRN2")
    x = nc.dram_tensor("x", [NWIN, ROWS_PER_WIN, CHUNK], mybir.dt.uint8, kind="ExternalInput")
    idx = nc.dram_tensor("idx", [P, NWIN * ICOL], mybir.dt.int16, kind="ExternalInput")
    mask = nc.dram_tensor("mask", [P, NWIN * SLOTS * CHUNK], mybir.dt.uint8, kind="ExternalInput")
    partial = nc.dram_tensor("partial", [P, 1], mybir.dt.float32, kind="ExternalOutput")

    idx_t = nc.alloc_sbuf_tensor("idx_t", [P, NWIN * ICOL], mybir.dt.int16)
    mask_t = nc.alloc_sbuf_tensor("mask_t", [P, NWIN * SLOTS * CHUNK], mybir.dt.uint8)
    gath = nc.alloc_sbuf_tensor("gath", [P, NWIN * SLOTS, CHUNK], mybir.dt.uint8)
    prod = nc.alloc_sbuf_tensor("prod", [P, NWIN * SLOTS * CHUNK], mybir.dt.bfloat16)
    red = nc.alloc_sbuf_tensor("red", [P, NWIN + 1], mybir.dt.float32)

    s_idx = nc.alloc_semaphore("s_idx")
    s_mask = nc.alloc_semaphore("s_mask")
    s_g = [nc.alloc_semaphore(f"s_g{w}") for w in range(NWIN)]
    s_v = nc.alloc_semaphore("s_v")
    s_out = nc.alloc_semaphore("s_out")

    # idx on Sync HWDGE (first-byte latency gates the first gather's Q7 gen);
    # mask on Scalar HWDGE, fully overlapped with the gather gens.
    nc.sync.dma_start(out=idx_t.ap(), in_=idx[:]).then_inc(s_idx, 16)
    nc.scalar.dma_start(out=mask_t.ap(), in_=mask[:]).then_inc(s_mask, 16)

    nc.gpsimd.wait_ge(s_idx, 16)
    for w in range(NWIN):
        nc.gpsimd.dma_gather(
            out_ap=gath.ap()[:, w * SLOTS : (w + 1) * SLOTS, :],
            in_ap=x[w],
            idxs_ap=idx_t.ap()[:, w * ICOL : (w + 1) * ICOL],
            num_idxs=NIDX,
            num_idxs_reg=NIDX,
            elem_size=CHUNK,
        ).then_inc(s_g[w], 16)

    # tensor_tensor_reduce bricks the exec unit on this HW (bf16 AND fp8,
    # NRT_EXEC_UNIT_UNRECOVERABLE) -- use the HW-verified 2-pass form:
    # per-window tensor_tensor multiply (fp8 x fp8 -> bf16, exact for a
    # 0/1 mask) then tensor_reduce (bf16 -> f32). DVE executes in order, so
    # only cross-engine deps need semaphores; windows 0-2 hide under the
    # later windows' Q7 descriptor gen.
    fp8 = mybir.dt.float8e4
    nc.vector.wait_ge(s_mask, 16)
    for w in range(NWIN):
        nc.vector.wait_ge(s_g[w], 16)
        sl = slice(w * SLOTS * CHUNK, (w + 1) * SLOTS * CHUNK)
        nc.vector.tensor_tensor(
            out=prod.ap()[:, sl],
            in0=gath.ap().rearrange("p s c -> p (s c)")[:, sl].bitcast(fp8),
            in1=mask_t.ap()[:, sl].bitcast(fp8),
            op=mybir.AluOpType.mult,
        )
        nc.vector.tensor_reduce(
            out=red.ap()[:, w : w + 1],
            in_=prod.ap()[:, sl],
            axis=mybir.AxisListType.X,
            op=mybir.AluOpType.add,
        )
    nc.vector.tensor_reduce(
        out=red.ap()[:, NWIN : NWIN + 1],
        in_=red.ap()[:, 0:NWIN],
        axis=mybir.AxisListType.X,
        op=mybir.AluOpType.add,
    ).then_inc(s_v, 1)

    nc.sync.wait_ge(s_v, 1)
    nc.sync.dma_start(out=partial[:], in_=red.ap()[:, NWIN : NWIN + 1]).then_inc(
        s_out, 16
    )

    nc.compile()
    return nc


def _get_nc():
    if "nc" not in _NC_CACHE:
        _NC_CACHE["nc"] = _build()
    return _NC_CACHE["nc"]


def _to_fp8_bytes(a):
    import ml_dtypes

    return np.asarray(a, dtype=np.float32).astype(ml_dtypes.float8_e4m3fn).view(np.uint8)


def _shard(output, targets):
    import ml_dtypes

    one_fp8 = np.float32(1.0).astype(ml_dtypes.float8_e4m3fn).view(np.uint8)

    xs = _to_fp8_bytes(output).reshape(NCORES, NL * C)
    pad = NWIN * WIN_BYTES - NL * C
    xs = np.concatenate([xs, np.zeros((NCORES, pad), np.uint8)], axis=1)
    xs = np.ascontiguousarray(xs.reshape(NCORES, NWIN, ROWS_PER_WIN, CHUNK))

    t = targets.reshape(NCORES, NL).astype(np.int64)
    addr = np.arange(NL, dtype=np.int64)[None, :] * C + t  # [NCORES, NL] byte addr
    win = addr >> 23
    row = (addr >> 8) & (ROWS_PER_WIN - 1)
    pos = addr & (CHUNK - 1)

    idxs = np.zeros((NCORES, NWIN, NIDX), np.int16)
    masks = np.zeros((NCORES, P, NWIN * SLOTS * CHUNK), np.uint8)
    for c in range(NCORES):
        for w in range(NWIN):
            sel = np.nonzero(win[c] == w)[0]
            cnt = sel.size
            if cnt > NIDX:
                raise RuntimeError(f"window overflow: core {c} win {w} has {cnt} > {NIDX}")
            idxs[c, w, :cnt] = row[c, sel]
            k = np.arange(cnt)
            p_slot, s_slot = k % P, k // P
            masks[c, p_slot, (w * SLOTS + s_slot) * CHUNK + pos[c, sel]] = one_fp8

    # wrap idx k -> [k%16, k//16], then replicate across the 8 Q7 cores
    wrapped = idxs.reshape(NCORES, NWIN, ICOL, 16).transpose(0, 3, 1, 2).reshape(NCORES, 16, NWIN * ICOL)
    wrapped = np.ascontiguousarray(np.tile(wrapped, (1, 8, 1)))
    return xs, wrapped, masks


def _run(output, targets, **kwargs):
    xs, idxs, masks = _shard(output, targets)
    in_maps = [
        {"x": xs[c], "idx": idxs[c], "mask": masks[c]} for c in range(NCORES)
    ]
    return run_bass_kernel_spmd(
        _get_nc(), in_maps, core_ids=list(range(NCORES)), **kwargs
    )


def kernel(output, targets):
    res = _run(output, targets)
    total = sum(float(r["partial"].sum()) for r in res.results)
    return np.array(np.float32(1.0) - np.float32(total / N), dtype=np.float32)


# revision 6
# speedup vs baseline: 2.1762x; 2.1762x over previous
"""Trainium2 Bass kernel for nn_CosineLoss: mean_i(1 - output[i, targets[i]]).

Strategy (data-parallel over the batch dim, 8 cores):
  - Core c owns rows [c*1024, (c+1)*1024) of `output` ([1024, 32000] f32 shard)
    plus flat element offsets idx[i] = i*32000 + targets[i] for its rows
    (int32, laid out [128, 8] in SBUF).
  - On device: 8 indirect DMAs (128 descriptors each -- the HW unrolls one
    descriptor per dest partition row and consumes ONE offset per row, so 128
    scattered elements per instruction is a hard cap; probed on HW) gather
    the 1024 needed f32 elements from HBM (4 KB instead of 131 MB).
    Q7 descriptor generation is the serial bottleneck: ~1.4us per SWDGE
    instruction (994ns fixed ucode prologue + dispatch), ~11us total.
    (InstDMAGatherAnt would do 512 offsets per instruction but its ext-isa
    ucode measures ~4.6us/call plus an ~8.5us one-time IRAM library load --
    strictly worse. Measured, not guessed.)
  - Output path: gpsimd issues a 9th SWDGE DMA right after the gathers that
    reads the gathered [128, 8] tile back to DRAM. Same qPoolDynamic ring
    and same partition->engine mapping as the gathers' SBUF writes, so each
    SDMA engine drains it strictly after its gather descriptors -- no
    completion semaphore, no DVE/PE/ACT chain on the critical path
    (saves ~1.2us of post-gather latency vs reduce-then-DMA).
  - Host sums the 8x1024 gathered values and returns 1 - total/8192.
"""

import numpy as np

from concourse import bacc, bass, mybir
from concourse.bass_utils import run_bass_kernel_spmd

N = 8192
C = 32000
NCORES = 8
NL = N // NCORES  # 1024 rows per core
P = 128
F = NL // P  # 8 gathered elements per partition

_NC_CACHE = {}


def _build():
    nc = bacc.Bacc("TRN2")
    x = nc.dram_tensor("x", [NL, C], mybir.dt.float32, kind="ExternalInput")
    idx = nc.dram_tensor("idx", [P, F], mybir.dt.int32, kind="ExternalInput")
    gout = nc.dram_tensor("gout", [P, F], mybir.dt.float32, kind="ExternalOutput")

    idx_t = nc.alloc_sbuf_tensor("idx_t", [P, F], mybir.dt.int32)
    gath = nc.alloc_sbuf_tensor("gath", [P, F], mybir.dt.float32)

    s_idx = nc.alloc_semaphore("s_idx")  # idx DMA completion (+16)
    s_g = nc.alloc_semaphore("s_g")  # gather DMA completions (+16 each)
    s_out = nc.alloc_semaphore("s_out")  # readback completion (unwaited; exit drain covers it)

    nc.sync.dma_start(out=idx_t.ap(), in_=idx[:]).then_inc(s_idx, 16)

    nc.gpsimd.wait_ge(s_idx, 16)
    for j in range(F):
        nc.gpsimd.indirect_dma_start(
            out=gath.ap()[:, j : j + 1],
            out_offset=None,
            in_=x[:],
            in_offset=bass.IndirectOffsetOnAxis(ap=idx_t.ap()[:, j : j + 1], axis=1),
        ).then_inc(s_g, 16)

    # ring-ordered readback: descriptors queue behind the gathers on the
    # same per-engine FIFOs, so this needs no wait on s_g.
    nc.gpsimd.dma_start(out=gout[:], in_=gath.ap()).then_inc(s_out, 16)

    nc.compile()
    return nc


def _get_nc():
    if "nc" not in _NC_CACHE:
        _NC_CACHE["nc"] = _build()
    return _NC_CACHE["nc"]


def _shard(output, targets):
    xs = np.ascontiguousarray(
        output.reshape(NCORES, NL, C).astype(np.float32, copy=False)
    )
    flat = np.arange(NL, dtype=np.int32) * C + targets.reshape(NCORES, NL).astype(
        np.int32
    )
    return xs, np.ascontiguousarray(flat.reshape(NCORES, P, F))


def _run(output, targets, **kwargs):
    xs, idx = _shard(output, targets)
    in_maps = [{"x": xs[c], "idx": idx[c]} for c in range(NCORES)]
    return run_bass_kernel_spmd(
        _get_nc(), in_maps, core_ids=list(range(NCORES)), **kwargs
    )


def kernel(output, targets):
    res = _run(output, targets)
    total = sum(float(r["gout"].sum(dtype=np.float64)) for r in res.results)
    return np.array(np.float32(1.0) - np.float32(total / N), dtype=np.float32)
